# revision 3
# baseline (speedup 1.0000x reference)
"""BitNet ternary linear layer on 8 Trainium2 NeuronCores — v6.

y = x @ (W * s)^T with x (32, 4096) f32, W (11008, 4096) ternary {-1,0,+1}.

Strategy (memory-bound; the HW floor is ~12.5us of fixed framework
lead-in + walrus sem-clear postamble, plus ~15us of W streaming at the
contended ~360-420 GB/s per-core HBM rate):
  - Tensor-parallel: W rows sharded 8 ways (1376 per core), x replicated.
  - Host prep folds s into x and splits it into 2 fp8 E4M3 planes
    (value ~= p0 + p1/16, rel err ~2e-3 vs the 2e-2 gate) stacked along
    the stationary dim: M = 64.  W is fp8 (exact for ternary), k-major
    per partition.
  - DMAs are issued FIRST: x + odd stripes on Sync's HWDGE ring, even
    stripes on Scalar's.  Stripes ramp small -> big -> small and the
    last pass ships as three column-sliced stripes so each output chunk
    closes (copy + out-DMA) as soon as its slice lands.
  - fp8 DoubleRow matmuls, K=256/pass, 16 passes.  Each chunk owns its
    OWN single-bank PSUM tile — with one shared tile, Tile serializes
    the chunk-0 PSUM->SBUF cast against the chunk-1 matmul (measured
    3.8us stall).
  - HAM clock gate: 8 warmup matmuls during the DMA lead-in, then
    per-pass filler matmuls that REUSE the pass's stationary operand
    (a second matmul on the same ldweights into a scratch bank — a
    different stationary would force a weight swap costing ~190ns/pass).
    Fillers stop after pass 8; past that the PE runs a backlog anyway.
  - bf16 partials out (halves the output DMA + copy time); the host
    applies the 1/16 plane scale and sums in f32.
"""

import numpy as np
import ml_dtypes

N_CORES = 8
B, I, O = 32, 4096, 11008
OC = O // N_CORES        # 1376
NP = I // 256            # 16 DoubleRow passes (K=256 each)
NSPLIT = 2               # fp8 planes of x
ALPHA = 16.0
M = NSPLIT * B           # 64 stationary columns
PASS_STRIPES = [(0, 1), (1, 2), (3, 2), (5, 2), (7, 2), (9, 2), (11, 2), (13, 1), (14, 1)]
OCHUNKS = [(0, 512), (512, 512), (1024, 352)]
WARMUP_MMS = 8
# filler moving-width per pass (0 = none): big early (stripe waits are
# 1-3us there), tapering off once the stream saturates.
FILLER_N = {0: (512, 512, 512), 1: (512, 512), 2: (512, 512)}
FILLER_N.update({j: (256,) for j in range(3, 9)})
FILLER_N.update({j: (128,) for j in range(9, 13)})

_BUILT = None


def _build():
    import concourse.bacc as bacc
    import concourse.mybir as mybir
    from concourse.tile import TileContext

    f8 = mybir.dt.float8e4
    nc = bacc.Bacc("TRN2", target_bir_lowering=False, debug=False)
    xt = nc.dram_tensor("xt", (128, NP * 2 * M), f8, kind="ExternalInput")
    wt = nc.dram_tensor("wt", (128, NP * 2 * OC), f8, kind="ExternalInput")
    yp = nc.dram_tensor("yp", (M, OC), mybir.dt.bfloat16, kind="ExternalOutput")

    with TileContext(nc) as tc:
        with (
            tc.tile_pool(name="xp", bufs=1) as xp,
            tc.tile_pool(name="wp", bufs=1) as wp,
            tc.tile_pool(name="pp", bufs=1, space="PSUM") as pp,
            tc.tile_pool(name="op", bufs=1) as op,
        ):
            # ---- DMAs first: nothing precedes them in any engine program.
            xs = xp.tile([128, NP * 2 * M], f8)
            nc.sync.dma_start(xs[:, :], xt[:, :])

            stripes = []
            for s, (p0, np_s) in enumerate(PASS_STRIPES):
                w = wp.tile([128, np_s * 2 * OC], f8, name=f"w{s}", tag=f"w{s}")
                o0 = p0 * 2 * OC
                eng = nc.scalar if s % 2 == 0 else nc.sync
                eng.dma_start(w[:, :], wt[:, o0 : o0 + np_s * 2 * OC])
                stripes.append(w)
            # last pass (15), column-sliced per output chunk
            wt4 = wt[:, :].rearrange("p (j i o) -> p j i o", j=NP, i=2, o=OC)
            lastw = []
            for c, (o0, n) in enumerate(OCHUNKS):
                w = wp.tile([128, 2, n], f8, name=f"wl{c}", tag=f"wl{c}")
                eng = nc.sync if c % 2 == 0 else nc.scalar
                eng.dma_start(w[:, :, :], wt4[:, NP - 1, :, o0 : o0 + n])
                lastw.append(w)

            # ---- PE warmup during the DMA lead-in (DVE memset: GpSimd /
            # Sync / Scalar programs stay DMA-only at the front).
            wsrc = xp.tile([128, 512], f8, name="wsrc")
            nc.vector.memset(wsrc[:, :], 0.0)
            scratch = pp.tile([128, 512], mybir.dt.float32, name="scratch")
            for _ in range(WARMUP_MMS):
                nc.tensor.matmul(
                    scratch[:, :], wsrc[:, 0:128], wsrc[:, 0:512],
                    start=True, stop=True,
                )

            # per-chunk single-bank PSUM accumulators + a DR filler bank
            psc = [
                pp.tile([M, n], mybir.dt.float32, name=f"ps{c}")
                for c, (_, n) in enumerate(OCHUNKS)
            ]
            fsc = pp.tile([M, 512], mybir.dt.float32, name="fsc")
            x4 = xs[:, :].rearrange("p (j i m) -> p j i m", j=NP, i=2, m=M)

            for s, (p0, np_s) in enumerate(PASS_STRIPES):
                w4 = stripes[s][:, :].rearrange(
                    "p (jj i o) -> p jj i o", jj=np_s, i=2, o=OC
                )
                for jj in range(np_s):
                    j = p0 + jj
                    for c, (o0, n) in enumerate(OCHUNKS):
                        nc.tensor.matmul(
                            psc[c][:, 0:n],
                            x4[:, j],
                            w4[:, jj, :, o0 : o0 + n],
                            start=(j == 0),
                            stop=False,
                            perf_mode=mybir.MatmulPerfMode.DoubleRow,
                        )
                    # fillers reuse pass j's stationary (no weight swap)
                    for fn in FILLER_N.get(j, ()):
                        nc.tensor.matmul(
                            fsc[:, 0:fn],
                            x4[:, j],
                            w4[:, jj, :, 0:fn],
                            start=True,
                            stop=True,
                            perf_mode=mybir.MatmulPerfMode.DoubleRow,
                        )
            # last pass: chunk-major; with per-chunk PSUM tiles the copies
            # and out-DMAs overlap the remaining matmuls.
            for c, (o0, n) in enumerate(OCHUNKS):
                nc.tensor.matmul(
                    psc[c][:, 0:n],
                    x4[:, NP - 1],
                    lastw[c][:, :, :],
                    start=False,
                    stop=True,
                    perf_mode=mybir.MatmulPerfMode.DoubleRow,
                )
                sb = op.tile([M, n], mybir.dt.bfloat16, name=f"sb{c}", tag=f"sb{c}")
                if c % 2 == 0:
                    nc.vector.tensor_copy(sb[:, :], psc[c][:, 0:n])
                else:
                    nc.scalar.copy(sb[:, :], psc[c][:, 0:n])
                eng = nc.sync if c % 2 == 0 else nc.scalar
                eng.dma_start(yp[:, o0 : o0 + n], sb[:, :])

    nc.finalize()
    return nc


def _get_nc():
    global _BUILT
    if _BUILT is None:
        _BUILT = _build()
    return _BUILT


def _fp8_split(v, nsplit):
    """Split v into fp8 planes: v ~= sum_q planes[q] / ALPHA**q."""
    planes = []
    rem = v.astype(np.float32)
    for q in range(nsplit):
        p = (rem * np.float32(ALPHA**q)).astype(ml_dtypes.float8_e4m3fn)
        planes.append(p)
        rem = rem - p.astype(np.float32) / np.float32(ALPHA**q)
    return planes


def _prep_inputs(x, weight, scale_factor):
    x = np.asarray(x, dtype=np.float32)
    weight = np.asarray(weight, dtype=np.float32)
    s = np.float32(np.asarray(scale_factor))

    xsT = (x * s).T.astype(np.float32)                  # [I, B]
    planes = _fp8_split(xsT, NSPLIT)
    stacked = np.concatenate(planes, axis=1)            # [I, M]
    xt = np.ascontiguousarray(
        stacked.reshape(NP, 2, 128, M).transpose(2, 0, 1, 3).reshape(128, NP * 2 * M)
    )

    in_maps = []
    for c in range(N_CORES):
        wc = weight[c * OC : (c + 1) * OC, :]           # [OC, I]
        wq = wc.T.astype(ml_dtypes.float8_e4m3fn)       # [I, OC], exact
        wtc = np.ascontiguousarray(
            wq.reshape(NP, 2, 128, OC).transpose(2, 0, 1, 3).reshape(128, NP * 2 * OC)
        )
        in_maps.append({"xt": xt, "wt": wtc})
    return in_maps


def _run(in_maps, trace=False, tmpdir=None):
    from concourse.bass_utils import run_bass_kernel_spmd

    return run_bass_kernel_spmd(
        _get_nc(), in_maps, core_ids=list(range(N_CORES)), trace=trace, tmpdir=tmpdir
    )


def _combine(yp):
    acc = yp[0:B].astype(np.float32).copy()
    for q in range(1, NSPLIT):
        acc += yp[q * B : (q + 1) * B].astype(np.float32) * np.float32(1.0 / ALPHA**q)
    return acc


def kernel(x, weight, scale_factor):
    in_maps = _prep_inputs(x, weight, scale_factor)
    try:
        res = _run(in_maps)
    except Exception:
        # transient runtime/device hiccups happen; one retry is cheap and
        # the output is still checked downstream
        res = _run(in_maps)
    return np.concatenate(
        [_combine(res.results[c]["yp"]) for c in range(N_CORES)], axis=1
    )
